# revision 1
# baseline (speedup 1.0000x reference)
"""GAT + global-max-pool + LSTM + Linear kernel for Trainium2 (8 NeuronCores).

Sharding: data-parallel over the batch axis B=8 -> one sequence b per core.
Each core computes the GAT over its 16 graphs (t=0..15), global-max-pools,
runs the LSTM over its sequence locally, and emits one [8] output row.

GAT aggregation strategy (per core, per graph g, head h):
  z[m, n]   = a_s[m] + a_d[n]                 (dense outer sum via PE broadcast)
  ex[m, n]  = exp(leaky_relu(z)) = max(exp(z), exp(0.2 z))   (two ACT Exp passes)
  A[m, n]   = ex * cnt[m, n]                  (cnt = host-built edge-count mask)
  out[n, f] = sum_m xp[m, f] A[m, n] ;  denom[n] = sum_m A[m, n]
  (single matmul per 128-node src block: lhsT = [xp_h | ones] -> 33 out rows)
  gat[n, f] = relu(out[n, f] / denom[n] + b_gat); pooled = max over n.

softmax max-subtraction is dropped: alpha = ex/sum(ex) is invariant to the
per-dst shift and fp32/bf16 exp() of |z| <~ 10 cannot overflow.
"""

import numpy as np

import concourse.bacc as bacc
import concourse.bass as bass
import concourse.mybir as mybir
import concourse.tile as tile
from concourse.bass_utils import run_bass_kernel_spmd

B, T, N, F_IN = 8, 16, 1000, 16
H, D = 4, 32
HD = H * D          # 128
HL = 64
OUT = 8
NEG = 0.2
NPAD = 1024         # padded node count
NBLK = 8            # src blocks of 128
G = T               # graphs per core

FP = mybir.dt.float32
BF = mybir.dt.bfloat16
AX = mybir.AxisListType
AF = mybir.ActivationFunctionType
OPS = mybir.AluOpType

_CACHE = {}


def _build_nc():
    nc = bacc.Bacc("TRN2", target_bir_lowering=False, debug=False)

    # ---- DRAM I/O ----
    d_x = nc.dram_tensor("x_in", [G * NPAD, F_IN], FP, kind="ExternalInput").ap()
    d_wgat = nc.dram_tensor("w_gat", [F_IN, HD], FP, kind="ExternalInput").ap()
    d_wa8 = nc.dram_tensor("w_a8", [F_IN, 128], FP, kind="ExternalInput").ap()
    d_cnt = nc.dram_tensor("cntmask", [128, NBLK * NPAD], BF, kind="ExternalInput").ap()
    d_ones = nc.dram_tensor("ones_row", [1, 128], BF, kind="ExternalInput").ap()
    d_ones32 = nc.dram_tensor("ones_c32", [33, 32], FP, kind="ExternalInput").ap()
    d_ident = nc.dram_tensor("ident", [128, 128], FP, kind="ExternalInput").ap()
    d_bgat = nc.dram_tensor("b_gat", [32, H], FP, kind="ExternalInput").ap()
    d_wih = nc.dram_tensor("wih_t", [HD, 4 * HL], FP, kind="ExternalInput").ap()
    d_whh = nc.dram_tensor("whh_t", [HL, 4 * HL], FP, kind="ExternalInput").ap()
    d_bls = nc.dram_tensor("b_lstm", [HL, 4], FP, kind="ExternalInput").ap()
    d_wclf = nc.dram_tensor("wclf_t", [HL, OUT], FP, kind="ExternalInput").ap()
    d_bclf = nc.dram_tensor("b_clf", [OUT, 1], FP, kind="ExternalInput").ap()
    d_y = nc.dram_tensor("y", [OUT, 1], FP, kind="ExternalOutput").ap()

    with tile.TileContext(nc) as tc:
        with (
            tc.tile_pool(name="const", bufs=1) as cpool,
            tc.tile_pool(name="stage", bufs=2) as spool,
            tc.tile_pool(name="edense", bufs=3) as epool,
            tc.tile_pool(name="small", bufs=2) as mpool,
            tc.tile_pool(name="lstm", bufs=2) as lpool,
            tc.tile_pool(name="ps_misc", bufs=2, space="PSUM") as ps_misc,
            tc.tile_pool(name="ps_out", bufs=1, space="PSUM") as ps_out,
            tc.tile_pool(name="ps_vb", bufs=1, space="PSUM") as ps_vb,
        ):
            # ---- load constants ----
            c_x = cpool.tile([128, G * NPAD * F_IN // 128], FP, tag="xall")  # [128, 2048]
            # x rows (t*1024+m); partition p holds rows {128*i+p}: col block i
            nc.sync.dma_start(
                c_x[:].rearrange("p (i f) -> p i f", f=F_IN),
                d_x.rearrange("(i p) f -> p i f", p=128),
            )
            c_wgat = cpool.tile([F_IN, HD], FP, tag="wgat")
            nc.sync.dma_start(c_wgat[:], d_wgat)
            # col 32h = W_ad[:,h]; col 32h+1 = W_as[:,h]; rest zero
            c_wa8 = cpool.tile([F_IN, 128], FP, tag="wa8")
            nc.sync.dma_start(c_wa8[:], d_wa8)
            c_cnt = cpool.tile([128, NBLK * NPAD], BF, tag="cnt")
            nc.sync.dma_start(c_cnt[:], d_cnt)
            c_id = cpool.tile([128, 128], FP, tag="ident")
            nc.sync.dma_start(c_id[:], d_ident)
            c_onesB = cpool.tile([65, 128], BF, tag="onesB")
            nc.sync.dma_start(c_onesB[0:1, :], d_ones)
            nc.sync.dma_start(c_onesB[32:33, :], d_ones)
            nc.sync.dma_start(c_onesB[64:65, :], d_ones)
            c_ones32 = cpool.tile([33, 32], FP, tag="ones32")
            nc.sync.dma_start(c_ones32[:], d_ones32)
            c_bgat = cpool.tile([32, H], FP, tag="bgat")
            nc.sync.dma_start(c_bgat[:], d_bgat)
            c_wih = cpool.tile([HD, 4 * HL], FP, tag="wih")
            nc.sync.dma_start(c_wih[:], d_wih)
            c_whh = cpool.tile([HL, 4 * HL], FP, tag="whh")
            nc.sync.dma_start(c_whh[:], d_whh)
            c_bls = cpool.tile([HL, 4], FP, tag="bls")
            nc.sync.dma_start(c_bls[:], d_bls)
            c_wclf = cpool.tile([HL, OUT], FP, tag="wclf")
            nc.sync.dma_start(c_wclf[:], d_wclf)
            c_bclf = cpool.tile([OUT, 1], FP, tag="bclf")
            nc.sync.dma_start(c_bclf[:], d_bclf)

            # persistent per-core accumulators
            # u = exp(a_s), u5 = exp(0.2 a_s) column tiles (scale operands);
            # col index = ((g*NBLK + J)*H + h); partition = m within block
            c_uT = cpool.tile([128, G * NBLK * H], FP, tag="uT")
            c_u5T = cpool.tile([128, G * NBLK * H], FP, tag="u5T")
            c_pool = cpool.tile([HD, G], FP, tag="pooled")
            c_ph = []
            for h in range(H):
                ph_tile = cpool.tile([32, G], FP, tag=f"pool{h}")
                c_ph.append(ph_tile)

            for g in range(G):
                # ---- stage A: transpose x_g -> xT [16, 1024] ----
                xT = spool.tile([F_IN, NPAD], FP, tag="xT")
                for j in range(NPAD // 128):
                    i = g * (NPAD // 128) + j   # global 128-row tile index
                    pT = ps_misc.tile([F_IN, 128], FP, tag="pm")
                    nc.tensor.transpose(
                        pT[:], c_x[:, i * F_IN:(i + 1) * F_IN], c_id[:]
                    )
                    nc.vector.tensor_copy(xT[:, j * 128:(j + 1) * 128], pT[:])

                # ---- stage B: xp^T = W_gat^T @ xT ; a8 = W_a8^T @ xT ----
                xpT = spool.tile([HD, NPAD], FP, tag="xpT")
                for half in range(2):
                    pm = ps_misc.tile([HD, 512], FP, tag="pm")
                    nc.tensor.matmul(
                        pm[:], c_wgat[:], xT[:, half * 512:(half + 1) * 512],
                        start=True, stop=True,
                    )
                    nc.vector.tensor_copy(xpT[:, half * 512:(half + 1) * 512], pm[:])
                # v = exp(a_d), v5 = exp(0.2 a_d); a_d for head h on partition 32h
                s8v = spool.tile([128, NPAD], BF, tag="s8v")
                s8v5 = spool.tile([128, NPAD], BF, tag="s8v5")
                for half in range(2):
                    pa = ps_misc.tile([128, 512], FP, tag="pm")
                    nc.tensor.matmul(
                        pa[:], c_wa8[:], xT[:, half * 512:(half + 1) * 512],
                        start=True, stop=True,
                    )
                    nc.scalar.activation(
                        s8v[:, half * 512:(half + 1) * 512], pa[:], AF.Exp,
                        scale=1.0,
                    )
                    nc.scalar.activation(
                        s8v5[:, half * 512:(half + 1) * 512], pa[:], AF.Exp,
                        scale=NEG,
                    )
                # head 3 lives on partition 96 (invalid matmul base): relocate
                s8x3 = spool.tile([1, NPAD], BF, tag="s8x3")
                nc.sync.dma_start(s8x3[:], s8v[96:97, :])
                s8x53 = spool.tile([1, NPAD], BF, tag="s8x53")
                nc.sync.dma_start(s8x53[:], s8v5[96:97, :])

                # xp33: per src block J: [xp_h | 1] column groups, bf16
                # layout [128, NBLK * (H*33)]; col = J*132 + h*33 + d (d<32), ones at h*33+32
                xp33 = spool.tile([128, NBLK * (H * 33 + 4)], BF, tag="xp33")
                for J in range(NBLK):
                    pX = ps_misc.tile([128, 128], FP, tag="pm")
                    nc.tensor.transpose(
                        pX[:], xpT[:, J * 128:(J + 1) * 128], c_id[:]
                    )
                    base = J * (H * 33 + 4)
                    # copy [128, 32] per head into strided slots
                    nc.vector.tensor_copy(
                        xp33[:, base:base + H * 33].rearrange(
                            "p (h q) -> p h q", q=33
                        )[:, :, 0:32],
                        pX[:].rearrange("p (h q) -> p h q", q=32),
                    )
                    nc.vector.memset(
                        xp33[:, base:base + H * 33].rearrange(
                            "p (h q) -> p h q", q=33
                        )[:, :, 32:33],
                        1.0,
                    )
                    # a_sT columns for this (g, J): [128 m, 128] = xT_blk^T @ W_a8
                    pS = ps_misc.tile([128, 128], FP, tag="pm")
                    nc.tensor.matmul(
                        pS[:], xT[:, J * 128:(J + 1) * 128], c_wa8[:],
                        start=True, stop=True,
                    )
                    col = (g * NBLK + J) * H
                    aps = pS[:].rearrange("p (h q) -> p h q", q=32)[:, :, 1:2]
                    nc.scalar.activation(
                        c_uT[:, col:col + H],
                        aps, AF.Exp, scale=1.0,
                    )
                    nc.scalar.activation(
                        c_u5T[:, col:col + H],
                        aps, AF.Exp, scale=NEG,
                    )

                # ---- stage C/D: dense attention + aggregation per head ----
                for h in range(H):
                    # broadcast v rows to all 128 partitions via PE ones-matmul
                    if h < 3:
                        r = 32 * h
                        vrow, v5row = s8v[r:r + 1, :], s8v5[r:r + 1, :]
                    else:
                        r = 0
                        vrow, v5row = s8x3[:], s8x53[:]
                    vB = ps_vb.tile([128, NPAD], FP, tag="vB")
                    v5B = ps_vb.tile([128, NPAD], FP, tag="v5B")
                    for half in range(2):
                        sl = slice(half * 512, (half + 1) * 512)
                        nc.tensor.matmul(
                            vB[:, sl], c_onesB[r:r + 1, :], vrow[:, sl],
                            start=True, stop=True,
                        )
                        nc.tensor.matmul(
                            v5B[:, sl], c_onesB[r:r + 1, :], v5row[:, sl],
                            start=True, stop=True,
                        )
                    oph = ps_out.tile([33, NPAD], FP, tag="oph")
                    for J in range(NBLK):
                        col = (g * NBLK + J) * H + h
                        # E = u[m] * v[n] = exp(a_s[m] + a_d[n])
                        tE = epool.tile([128, NPAD], BF, tag="tE")
                        nc.scalar.activation(
                            tE[:], vB[:], AF.Copy,
                            bias=0.0, scale=c_uT[:, col:col + 1],
                        )
                        # M = max(E, u5[m] * v5[n]) = exp(lrelu(z))
                        tM = epool.tile([128, NPAD], BF, tag="tM")
                        nc.vector.scalar_tensor_tensor(
                            tM[:], v5B[:], c_u5T[:, col:col + 1], tE[:],
                            OPS.mult, OPS.max,
                        )
                        tA = epool.tile([128, NPAD], BF, tag="tA")
                        eng = nc.vector if J < 5 else nc.gpsimd
                        eng.tensor_tensor(
                            tA[:], tM[:], c_cnt[:, J * NPAD:(J + 1) * NPAD], OPS.mult
                        )
                        base = J * (H * 33 + 4) + h * 33
                        for half in range(2):
                            nc.tensor.matmul(
                                oph[:, half * 512:(half + 1) * 512],
                                xp33[:, base:base + 33],
                                tA[:, half * 512:(half + 1) * 512],
                                start=(J == 0), stop=(J == NBLK - 1),
                            )
                    # ---- divide by denom, relu, max-pool ----
                    # reciprocal of the denom row, folded 1x1024 -> 32x32 so
                    # the DVE reciprocal runs 32 partitions wide
                    denr = mpool.tile([33, NPAD], FP, tag="denr")
                    nc.scalar.copy(denr[32:33, :], oph[32:33, :])
                    den32 = mpool.tile([32, 32], FP, tag="den32")
                    nc.sync.dma_start(den32[:], denr[32:33, :])
                    rec32 = mpool.tile([32, 32], FP, tag="rec32")
                    nc.vector.reciprocal(rec32[:], den32[:])
                    rech = mpool.tile([33, NPAD], FP, tag="rech")
                    nc.sync.dma_start(rech[32:33, :], rec32[:])
                    outh = mpool.tile([32, NPAD], FP, tag="outh")
                    nc.vector.tensor_copy(outh[:], oph[0:32, :])
                    odiv = mpool.tile([32, NPAD], FP, tag="odiv")
                    for half in range(2):
                        rb = ps_misc.tile([32, 512], FP, tag="pm")
                        nc.tensor.matmul(
                            rb[:],
                            c_ones32[32:33, :],
                            rech[32:33, half * 512:(half + 1) * 512],
                            start=True, stop=True,
                        )
                        nc.vector.tensor_tensor(
                            odiv[:, half * 512:(half + 1) * 512],
                            outh[:, half * 512:(half + 1) * 512],
                            rb[:], OPS.mult,
                        )
                    orel = mpool.tile([32, NPAD], FP, tag="orel")
                    nc.vector.tensor_scalar(
                        orel[:], odiv[:], c_bgat[:, h:h + 1], 0.0,
                        OPS.add, OPS.max,
                    )
                    nc.vector.tensor_reduce(
                        c_ph[h][:, g:g + 1], orel[:, 0:N], AX.X, OPS.max
                    )

            # assemble pooled [128, G] from the four per-head tiles (DMA: cross-base)
            for h in range(H):
                nc.sync.dma_start(c_pool[h * 32:(h + 1) * 32, :], c_ph[h][:])

            # ---- LSTM over T steps ----
            # h is stored as h2 = 2h (W_hh/W_clf pre-halved on host);
            # c is stored as c2 = 2c (tanh applied with scale=0.5).
            hprev = lpool.tile([HL, 1], FP, tag="h0")
            cprev = lpool.tile([HL, 1], FP, tag="c0")
            nc.vector.memset(hprev[:], 0.0)
            nc.vector.memset(cprev[:], 0.0)
            for t in range(T):
                # four [64,1] gate psums (i, f, g, o), all base partition 0
                tga = []
                for gate in range(4):
                    psg = ps_misc.tile([HL, 1], FP, tag="pm")
                    nc.tensor.matmul(
                        psg[:], c_wih[:, gate * HL:(gate + 1) * HL],
                        c_pool[:, t:t + 1], start=True, stop=False,
                    )
                    nc.tensor.matmul(
                        psg[:], c_whh[:, gate * HL:(gate + 1) * HL],
                        hprev[:], start=False, stop=True,
                    )
                    tgt = lpool.tile([HL, 1], FP, tag=f"tg{gate}")
                    # gates i,f,o: sigmoid via tanh-half; gate g: plain tanh
                    sc = 1.0 if gate == 2 else 0.5
                    nc.scalar.activation(
                        tgt[:], psg[:], AF.Tanh,
                        bias=c_bls[:, gate:gate + 1], scale=sc,
                    )
                    tga.append(tgt)
                ti, tf, tg_, to = tga
                # v1 = (tf+1)*c2 = 4*sig(f)*c ; v2 = (ti+1)*tg = 2*sig(i)*g
                # c2_new = 2c_new = v1/2 + v2
                v1 = lpool.tile([HL, 1], FP, tag="v1")
                nc.vector.scalar_tensor_tensor(
                    v1[:], tf[:], 1.0, cprev[:], OPS.add, OPS.mult
                )
                v2 = lpool.tile([HL, 1], FP, tag="v2")
                nc.vector.scalar_tensor_tensor(
                    v2[:], ti[:], 1.0, tg_[:], OPS.add, OPS.mult
                )
                cnew = lpool.tile([HL, 1], FP, tag="c0")
                nc.vector.scalar_tensor_tensor(
                    cnew[:], v1[:], 0.5, v2[:], OPS.mult, OPS.add
                )
                tcn = lpool.tile([HL, 1], FP, tag="tcn")
                nc.scalar.activation(tcn[:], cnew[:], AF.Tanh, scale=0.5)
                hnew = lpool.tile([HL, 1], FP, tag="h0")
                # h2 = (to + 1) * tanh(c)
                nc.vector.scalar_tensor_tensor(
                    hnew[:], to[:], 1.0, tcn[:], OPS.add, OPS.mult
                )
                hprev, cprev = hnew, cnew

            ps3 = ps_misc.tile([OUT, 1], FP, tag="pm")
            nc.tensor.matmul(ps3[:], c_wclf[:], hprev[:], start=True, stop=True)
            ysb = lpool.tile([OUT, 1], FP, tag="ysb")
            nc.vector.tensor_tensor(ysb[:], ps3[:], c_bclf[:], OPS.add)
            nc.sync.dma_start(d_y, ysb[:])

    nc.compile()
    return nc


def _host_prep(inputs):
    x = np.asarray(inputs["x"], dtype=np.float32)          # [B, T, N, F]
    ei = np.asarray(inputs["edge_index"])
    W_gat = np.asarray(inputs["W_gat"], dtype=np.float32)  # [16, 128]
    att_src = np.asarray(inputs["att_src"], dtype=np.float32)  # [H, D]
    att_dst = np.asarray(inputs["att_dst"], dtype=np.float32)
    b_gat = np.asarray(inputs["b_gat"], dtype=np.float32)
    W_ih = np.asarray(inputs["W_ih"], dtype=np.float32)    # [256, 128]
    W_hh = np.asarray(inputs["W_hh"], dtype=np.float32)    # [256, 64]
    b_ih = np.asarray(inputs["b_ih"], dtype=np.float32)
    b_hh = np.asarray(inputs["b_hh"], dtype=np.float32)
    W_clf = np.asarray(inputs["W_clf"], dtype=np.float32)  # [8, 64]
    b_clf = np.asarray(inputs["b_clf"], dtype=np.float32)

    bf16 = mybir.dt.np(BF)

    # fold attention vectors: a_s = x @ (W_gat-reshaped @ att_src)
    Wr = W_gat.reshape(F_IN, H, D)
    W_as = np.einsum("fhd,hd->fh", Wr, att_src)            # [16, 4]
    W_ad = np.einsum("fhd,hd->fh", Wr, att_dst)
    w_a8 = np.zeros((F_IN, 128), dtype=np.float32)
    w_a8[:, 32 * np.arange(H)] = W_ad                      # a_d -> partition 32h
    w_a8[:, 32 * np.arange(H) + 1] = W_as                  # a_s -> 32h+1

    # edge counts with self loops, dense [1024, 1024]
    src = ei[0].astype(np.int64)
    dst = ei[1].astype(np.int64)
    Cm = np.zeros((NPAD, NPAD), dtype=np.float32)
    np.add.at(Cm, (src, dst), 1.0)
    Cm[np.arange(N), np.arange(N)] += 1.0                  # self loops
    Cm[NPAD - 1, N:] = 1.0  # dummy edges: keep pad-column denominators finite
    cntmask = (
        Cm.reshape(NBLK, 128, NPAD).transpose(1, 0, 2).reshape(128, NBLK * NPAD)
    ).astype(bf16)

    # x padded per core: [T, NPAD, F] flattened
    xpad = np.zeros((B, T, NPAD, F_IN), dtype=np.float32)
    xpad[:, :, :N, :] = x
    xcore = [xpad[b].reshape(T * NPAD, F_IN).copy() for b in range(B)]

    b_gates = (b_ih + b_hh).astype(np.float32)             # [256]
    bls = np.zeros((HL, 4), dtype=np.float32)
    bls[:, 0] = 0.5 * b_gates[0:64]                        # i (tanh-half trick)
    bls[:, 1] = 0.5 * b_gates[64:128]                      # f
    bls[:, 2] = b_gates[128:192]                           # g
    bls[:, 3] = 0.5 * b_gates[192:256]                     # o

    common = {
        "w_gat": W_gat,
        "ones_row": np.ones((1, 128), dtype=bf16),
        "ones_c32": np.ones((33, 32), dtype=np.float32),
        "w_a8": w_a8,
        "cntmask": cntmask,
        "ident": np.eye(128, dtype=np.float32),
        "b_gat": np.ascontiguousarray(b_gat.reshape(H, 32).T),
        "wih_t": np.ascontiguousarray(W_ih.T),             # [128, 256]
        "whh_t": np.ascontiguousarray(0.5 * W_hh.T),       # [64, 256] (h2 comp)
        "b_lstm": bls,
        "wclf_t": np.ascontiguousarray(0.5 * W_clf.T),     # [64, 8] (h2 comp)
        "b_clf": b_clf.reshape(OUT, 1),
    }
    in_maps = []
    for b in range(B):
        m = dict(common)
        m["x_in"] = xcore[b]
        in_maps.append(m)
    return in_maps


def kernel(**inputs):
    if "nc" not in _CACHE:
        _CACHE["nc"] = _build_nc()
    nc = _CACHE["nc"]
    in_maps = _host_prep(inputs)
    res = run_bass_kernel_spmd(nc, in_maps, core_ids=list(range(B)))
    y = np.stack([r["y"][:, 0] for r in res.results], axis=0)
    return y.astype(np.float32)


if __name__ == "__main__":
    import reference as R

    inp = R.setup_inputs()
    inp = {k: np.asarray(v) for k, v in inp.items()}
    out = kernel(**inp)
    print(out)



# revision 3
# speedup vs baseline: 1.3463x; 1.3463x over previous
"""GAT + global-max-pool + LSTM + Linear kernel for Trainium2 (8 NeuronCores), v2.

Sharding: data-parallel over batch B=8 -> one sequence b per core.

GAT reformulation (exact, per graph g, head h):
  softmax over in-edges of dst n is invariant to any per-column scale, so
  divide the attention matrix by u[m]*v[n] (u=exp(a_s), v=exp(a_d)):
    A~[m,n] = max(rho[m]*y[n], 1) * C[m,n]
  with rho = exp(-0.8*a_s), y = exp(-0.8*a_d), C = edge-count mask.
  The row factor u[m] folds into the aggregation lhsT (xpu = xp*u, u in the
  33rd denominator slot), the column factor v[n] cancels in num/den.

  Per-tile routes (tile = [128 src x 1024 dst], 8 per (g,h)):
   R4 : R = ACT-Relu(yB*rho - 1) ; RC = R*C (DVE TT) ; PE aggregates RC and C
   R4G: same but RC on GpSimd
   R1 : A~ = DVE TS max(yB*rho, 1) ; tA = A~*C (DVE TT) ; PE aggregates tA
  Epilogue uses max_n relu(x) = relu(max_n x) to pool before bias+relu.
"""

import numpy as np

import concourse.bacc as bacc
import concourse.bass as bass
import concourse.mybir as mybir
import concourse.tile as tile
from concourse.bass_utils import run_bass_kernel_spmd

B, T, N, F_IN = 8, 16, 1000, 16
H, D = 4, 32
HD = H * D          # 128
HL = 64
OUT = 8
NPAD = 1024
NBLK = 8
G = T

FP = mybir.dt.float32
BF = mybir.dt.bfloat16
AX = mybir.AxisListType
AF = mybir.ActivationFunctionType
OPS = mybir.AluOpType

# route per (h, J): 4=R4 (DVE mask), 5=R4G (GpSimd mask), 1=R1 (DVE TS+TT)
ROUTE = [
    [5, 4, 1, 5, 4, 5, 1, 4],
    [4, 5, 4, 1, 5, 4, 5, 5],
    [5, 1, 4, 5, 4, 5, 4, 1],
    [4, 5, 5, 4, 1, 5, 4, 1],
]

_CACHE = {}


def _build_nc():
    nc = bacc.Bacc("TRN2", target_bir_lowering=False, debug=False)

    # ---- DRAM I/O ----
    d_xt = nc.dram_tensor("x_t", [F_IN, G * NPAD], BF, kind="ExternalInput").ap()
    d_wgat = nc.dram_tensor("w_gat", [F_IN, HD], BF, kind="ExternalInput").ap()
    d_was = nc.dram_tensor("w_as", [F_IN, H], BF, kind="ExternalInput").ap()
    d_wad = nc.dram_tensor("w_ad", [F_IN, H], BF, kind="ExternalInput").ap()
    d_cnt = nc.dram_tensor("cntmask", [128, NBLK * NPAD], BF, kind="ExternalInput").ap()
    d_ones = nc.dram_tensor("ones65", [65, 128], BF, kind="ExternalInput").ap()
    d_bgat = nc.dram_tensor("b_gat", [32, H], FP, kind="ExternalInput").ap()
    d_wih = nc.dram_tensor("wih_t", [HD, 4 * HL], FP, kind="ExternalInput").ap()
    d_whh = nc.dram_tensor("whh_t", [HL, 4 * HL], FP, kind="ExternalInput").ap()
    d_bls = nc.dram_tensor("b_lstm", [HL, 4], FP, kind="ExternalInput").ap()
    d_wclf = nc.dram_tensor("wclf_t", [HL, OUT], FP, kind="ExternalInput").ap()
    d_bclf = nc.dram_tensor("b_clf", [OUT, 1], FP, kind="ExternalInput").ap()
    d_y = nc.dram_tensor("y", [OUT, 1], FP, kind="ExternalOutput").ap()

    with tile.TileContext(nc) as tc:
        with (
            tc.tile_pool(name="const", bufs=1) as cpool,
            tc.tile_pool(name="stage", bufs=2) as spool,
            tc.tile_pool(name="edense", bufs=4) as epool,
            tc.tile_pool(name="small", bufs=2) as mpool,
            tc.tile_pool(name="lstm", bufs=2) as lpool,
            tc.tile_pool(name="ps_misc", bufs=2, space="PSUM") as ps_misc,
            tc.tile_pool(name="ps_out", bufs=2, space="PSUM") as ps_out,
            tc.tile_pool(name="ps_y", bufs=1, space="PSUM") as ps_y,
        ):
            # ---- constants ----
            c_xT = cpool.tile([F_IN, G * NPAD], BF, tag="xT")
            nc.sync.dma_start(c_xT[:], d_xt)
            c_wgat = cpool.tile([F_IN, HD], BF, tag="wgat")
            nc.sync.dma_start(c_wgat[:], d_wgat)
            c_was = cpool.tile([F_IN, H], BF, tag="was")
            nc.sync.dma_start(c_was[:], d_was)
            c_wad = cpool.tile([F_IN, H], BF, tag="wad")
            nc.sync.dma_start(c_wad[:], d_wad)
            c_cnt = cpool.tile([128, NBLK * NPAD], BF, tag="cnt")
            nc.sync.dma_start(c_cnt[:], d_cnt)
            c_onesB = cpool.tile([65, 128], BF, tag="onesB")
            nc.sync.dma_start(c_onesB[:], d_ones)
            c_bgat = cpool.tile([32, H], FP, tag="bgat")
            nc.sync.dma_start(c_bgat[:], d_bgat)
            c_wih = cpool.tile([HD, 4 * HL], FP, tag="wih")
            nc.sync.dma_start(c_wih[:], d_wih)
            c_whh = cpool.tile([HL, 4 * HL], FP, tag="whh")
            nc.sync.dma_start(c_whh[:], d_whh)
            c_bls = cpool.tile([HL, 4], FP, tag="bls")
            nc.sync.dma_start(c_bls[:], d_bls)
            c_wclf = cpool.tile([HL, OUT], FP, tag="wclf")
            nc.sync.dma_start(c_wclf[:], d_wclf)
            c_bclf = cpool.tile([OUT, 1], FP, tag="bclf")
            nc.sync.dma_start(c_bclf[:], d_bclf)

            c_neg1 = cpool.tile([128, 1], FP, tag="neg1")
            nc.vector.memset(c_neg1[:], -1.0)
            c_ones32 = cpool.tile([33, 32], FP, tag="ones32")
            nc.vector.memset(c_ones32[:], 1.0)
            c_pool = cpool.tile([HD, G], FP, tag="pooled")
            c_ph = []
            for h in range(H):
                ph_tile = cpool.tile([32, G], FP, tag=f"pool{h}")
                c_ph.append(ph_tile)

            for g in range(G):
                xg = c_xT[:, g * NPAD:(g + 1) * NPAD]   # [16, 1024] bf16

                # ---- per-g stage: a_s cols (u, rho), a_d rows (y), xpu33 ----
                pS = ps_misc.tile([128, 4 * NBLK], FP, tag="pm")
                for J in range(NBLK):
                    nc.tensor.matmul(
                        pS[:, J * 4:(J + 1) * 4],
                        xg[:, J * 128:(J + 1) * 128], c_was[:],
                        start=True, stop=True,
                    )
                c_u = spool.tile([128, 4 * NBLK], FP, tag="ucols")
                nc.scalar.activation(c_u[:], pS[:], AF.Exp, scale=1.0)
                c_rho = spool.tile([128, 4 * NBLK], FP, tag="rhocols")
                nc.scalar.activation(c_rho[:], pS[:], AF.Exp, scale=-0.8)

                y4 = spool.tile([4, NPAD], BF, tag="y4")
                for half in range(2):
                    pAd = ps_misc.tile([4, 512], FP, tag="pm")
                    nc.tensor.matmul(
                        pAd[:],
                        c_wad[:], xg[:, half * 512:(half + 1) * 512],
                        start=True, stop=True,
                    )
                    nc.scalar.activation(
                        y4[:, half * 512:(half + 1) * 512], pAd[:],
                        AF.Exp, scale=-0.8,
                    )
                # relocate rows to matmul-legal base partitions 0/32/64 (+spare)
                y65 = spool.tile([65, NPAD], BF, tag="y65")
                nc.sync.dma_start(y65[0:1, :], y4[0:1, :])
                nc.sync.dma_start(y65[32:33, :], y4[1:2, :])
                nc.sync.dma_start(y65[64:65, :], y4[2:3, :])
                y3x = spool.tile([1, NPAD], BF, tag="y3x")
                nc.sync.dma_start(y3x[:], y4[3:4, :])

                # xpu33: [128, J*132 + h*33 + (0..31 feats, 32 = u)]
                xpu33 = spool.tile([128, NBLK * 132], BF, tag="xpu33")
                for J in range(NBLK):
                    pX = ps_misc.tile([128, HD], FP, tag="pm")
                    nc.tensor.matmul(
                        pX[:], xg[:, J * 128:(J + 1) * 128], c_wgat[:],
                        start=True, stop=True,
                    )
                    base = J * 132
                    for h in range(H):
                        nc.vector.tensor_scalar(
                            xpu33[:, base + h * 33:base + h * 33 + 32],
                            pX[:, h * 32:(h + 1) * 32],
                            c_u[:, J * 4 + h:J * 4 + h + 1], 0.0,
                            OPS.mult, OPS.add,
                        )
                    nc.vector.tensor_copy(
                        xpu33[:, base:base + 132].rearrange(
                            "p (h q) -> p h q", q=33
                        )[:, :, 32:33],
                        c_u[:, J * 4:(J + 1) * 4].rearrange("p (h q) -> p h q", q=1),
                    )

                # ---- hot loop: per (h): yB bcast + 8 J tiles + epilogue ----
                for h in range(H):
                    if h < 3:
                        r = 32 * h
                        yrow = y65[r:r + 1, :]
                    else:
                        r = 0
                        yrow = y3x[:]
                    yB = ps_y.tile([128, NPAD], FP, tag="yB")
                    for half in range(2):
                        sl = slice(half * 512, (half + 1) * 512)
                        nc.tensor.matmul(
                            yB[:, sl], c_onesB[r:r + 1, :], yrow[:, sl],
                            start=True, stop=True,
                        )
                    oph = ps_out.tile([33, NPAD], FP, tag="oph")
                    # matmul bookkeeping: contributions per half
                    n_mm = sum(2 if ROUTE[h][J] != 1 else 1 for J in range(NBLK))
                    mm_i = 0
                    for J in range(NBLK):
                        rt = ROUTE[h][J]
                        rho_col = c_rho[:, J * 4 + h:J * 4 + h + 1]
                        cslice = c_cnt[:, J * NPAD:(J + 1) * NPAD]
                        lhs = xpu33[:, J * 132 + h * 33:J * 132 + h * 33 + 33]
                        if rt == 1:
                            tA = epool.tile([128, NPAD], BF, tag="tA")
                            tAh = epool.tile([128, NPAD], BF, tag="tAh")
                            nc.vector.tensor_scalar(
                                tAh[:], yB[:], rho_col, 1.0, OPS.mult, OPS.max
                            )
                            nc.vector.tensor_tensor(tA[:], tAh[:], cslice, OPS.mult)
                            rhss = [tA[:]]
                        else:
                            R = epool.tile([128, NPAD], BF, tag="R")
                            nc.scalar.activation(
                                R[:], yB[:], AF.Relu, bias=c_neg1[:], scale=rho_col
                            )
                            RC = epool.tile([128, NPAD], BF, tag="RC")
                            eng = nc.vector if rt == 4 else nc.gpsimd
                            eng.tensor_tensor(RC[:], R[:], cslice, OPS.mult)
                            rhss = [RC[:], cslice]
                        for rhs in rhss:
                            for half in range(2):
                                sl = slice(half * 512, (half + 1) * 512)
                                nc.tensor.matmul(
                                    oph[:, sl], lhs, rhs[:, sl],
                                    start=(mm_i == 0), stop=(mm_i == n_mm - 1),
                                )
                            mm_i += 1

                    # ---- epilogue: r = 1/den, od = num*r, maxpool, bias+relu ----
                    denr = mpool.tile([33, NPAD], FP, tag="denr")
                    nc.scalar.copy(denr[32:33, :], oph[32:33, :])
                    den32 = mpool.tile([32, 32], FP, tag="den32")
                    nc.sync.dma_start(den32[:], denr[32:33, :])
                    rec32 = mpool.tile([32, 32], FP, tag="rec32")
                    nc.vector.reciprocal(rec32[:], den32[:])
                    rech = mpool.tile([33, NPAD], FP, tag="rech")
                    nc.sync.dma_start(rech[32:33, :], rec32[:])
                    oph16 = mpool.tile([32, NPAD], BF, tag="oph16")
                    nc.scalar.copy(oph16[:], oph[0:32, :])
                    od = mpool.tile([32, NPAD], BF, tag="od")
                    for half in range(2):
                        sl = slice(half * 512, (half + 1) * 512)
                        rb = ps_misc.tile([32, 512], FP, tag="pm")
                        nc.tensor.matmul(
                            rb[:], c_ones32[32:33, :], rech[32:33, sl],
                            start=True, stop=True,
                        )
                        nc.vector.tensor_tensor(
                            od[:, sl], oph16[:, sl], rb[:], OPS.mult
                        )
                    trout = mpool.tile([32, 1], FP, tag="trout")
                    nc.vector.tensor_reduce(trout[:], od[:, 0:N], AX.X, OPS.max)
                    nc.vector.tensor_scalar(
                        c_ph[h][:, g:g + 1], trout[:],
                        c_bgat[:, h:h + 1], 0.0, OPS.add, OPS.max,
                    )

            # assemble pooled [128, G]
            for h in range(H):
                nc.sync.dma_start(c_pool[h * 32:(h + 1) * 32, :], c_ph[h][:])

            # ---- LSTM over T steps (identical to baseline) ----
            hprev = lpool.tile([HL, 1], FP, tag="h0")
            cprev = lpool.tile([HL, 1], FP, tag="c0")
            nc.vector.memset(hprev[:], 0.0)
            nc.vector.memset(cprev[:], 0.0)
            for t in range(T):
                tga = []
                for gate in range(4):
                    psg = ps_misc.tile([HL, 1], FP, tag="pm")
                    nc.tensor.matmul(
                        psg[:], c_wih[:, gate * HL:(gate + 1) * HL],
                        c_pool[:, t:t + 1], start=True, stop=False,
                    )
                    nc.tensor.matmul(
                        psg[:], c_whh[:, gate * HL:(gate + 1) * HL],
                        hprev[:], start=False, stop=True,
                    )
                    tgt = lpool.tile([HL, 1], FP, tag=f"tg{gate}")
                    sc = 1.0 if gate == 2 else 0.5
                    nc.scalar.activation(
                        tgt[:], psg[:], AF.Tanh,
                        bias=c_bls[:, gate:gate + 1], scale=sc,
                    )
                    tga.append(tgt)
                ti, tf, tg_, to = tga
                v1 = lpool.tile([HL, 1], FP, tag="v1")
                nc.vector.scalar_tensor_tensor(
                    v1[:], tf[:], 1.0, cprev[:], OPS.add, OPS.mult
                )
                v2 = lpool.tile([HL, 1], FP, tag="v2")
                nc.vector.scalar_tensor_tensor(
                    v2[:], ti[:], 1.0, tg_[:], OPS.add, OPS.mult
                )
                cnew = lpool.tile([HL, 1], FP, tag="c0")
                nc.vector.scalar_tensor_tensor(
                    cnew[:], v1[:], 0.5, v2[:], OPS.mult, OPS.add
                )
                tcn = lpool.tile([HL, 1], FP, tag="tcn")
                nc.scalar.activation(tcn[:], cnew[:], AF.Tanh, scale=0.5)
                hnew = lpool.tile([HL, 1], FP, tag="h0")
                nc.vector.scalar_tensor_tensor(
                    hnew[:], to[:], 1.0, tcn[:], OPS.add, OPS.mult
                )
                hprev, cprev = hnew, cnew

            ps3 = ps_misc.tile([OUT, 1], FP, tag="pm")
            nc.tensor.matmul(ps3[:], c_wclf[:], hprev[:], start=True, stop=True)
            ysb = lpool.tile([OUT, 1], FP, tag="ysb")
            nc.vector.tensor_tensor(ysb[:], ps3[:], c_bclf[:], OPS.add)
            nc.sync.dma_start(d_y, ysb[:])

    nc.compile()
    return nc


def _host_prep(inputs):
    x = np.asarray(inputs["x"], dtype=np.float32)
    ei = np.asarray(inputs["edge_index"])
    W_gat = np.asarray(inputs["W_gat"], dtype=np.float32)
    att_src = np.asarray(inputs["att_src"], dtype=np.float32)
    att_dst = np.asarray(inputs["att_dst"], dtype=np.float32)
    b_gat = np.asarray(inputs["b_gat"], dtype=np.float32)
    W_ih = np.asarray(inputs["W_ih"], dtype=np.float32)
    W_hh = np.asarray(inputs["W_hh"], dtype=np.float32)
    b_ih = np.asarray(inputs["b_ih"], dtype=np.float32)
    b_hh = np.asarray(inputs["b_hh"], dtype=np.float32)
    W_clf = np.asarray(inputs["W_clf"], dtype=np.float32)
    b_clf = np.asarray(inputs["b_clf"], dtype=np.float32)

    bf16 = mybir.dt.np(BF)

    Wr = W_gat.reshape(F_IN, H, D)
    W_as = np.einsum("fhd,hd->fh", Wr, att_src)
    W_ad = np.einsum("fhd,hd->fh", Wr, att_dst)

    src = ei[0].astype(np.int64)
    dst = ei[1].astype(np.int64)
    Cm = np.zeros((NPAD, NPAD), dtype=np.float32)
    np.add.at(Cm, (src, dst), 1.0)
    Cm[np.arange(N), np.arange(N)] += 1.0
    Cm[NPAD - 1, N:] = 1.0
    cntmask = (
        Cm.reshape(NBLK, 128, NPAD).transpose(1, 0, 2).reshape(128, NBLK * NPAD)
    ).astype(bf16)

    xpad = np.zeros((B, T, NPAD, F_IN), dtype=np.float32)
    xpad[:, :, :N, :] = x
    # [F, T*NPAD] per core
    xtcore = [
        np.ascontiguousarray(
            xpad[b].reshape(T * NPAD, F_IN).T
        ).astype(bf16)
        for b in range(B)
    ]

    b_gates = (b_ih + b_hh).astype(np.float32)
    bls = np.zeros((HL, 4), dtype=np.float32)
    bls[:, 0] = 0.5 * b_gates[0:64]
    bls[:, 1] = 0.5 * b_gates[64:128]
    bls[:, 2] = b_gates[128:192]
    bls[:, 3] = 0.5 * b_gates[192:256]

    common = {
        "w_gat": W_gat.astype(bf16),
        "w_as": W_as.astype(bf16),
        "w_ad": W_ad.astype(bf16),
        "cntmask": cntmask,
        "ones65": np.ones((65, 128), dtype=bf16),
        "b_gat": np.ascontiguousarray(b_gat.reshape(H, 32).T),
        "wih_t": np.ascontiguousarray(W_ih.T),
        "whh_t": np.ascontiguousarray(0.5 * W_hh.T),
        "b_lstm": bls,
        "wclf_t": np.ascontiguousarray(0.5 * W_clf.T),
        "b_clf": b_clf.reshape(OUT, 1),
    }
    in_maps = []
    for b in range(B):
        m = dict(common)
        m["x_t"] = xtcore[b]
        in_maps.append(m)
    return in_maps


def kernel(**inputs):
    if "nc" not in _CACHE:
        _CACHE["nc"] = _build_nc()
    nc = _CACHE["nc"]
    in_maps = _host_prep(inputs)
    res = run_bass_kernel_spmd(nc, in_maps, core_ids=list(range(B)))
    y = np.stack([r["y"][:, 0] for r in res.results], axis=0)
    return y.astype(np.float32)


if __name__ == "__main__":
    import reference as R

    inp = R.setup_inputs()
    inp = {k: np.asarray(v) for k, v in inp.items()}
    out = kernel(**inp)
    print(out)


# revision 4
# speedup vs baseline: 1.4301x; 1.0622x over previous
"""GAT + global-max-pool + LSTM + Linear kernel for Trainium2 (8 NeuronCores), v2.

Sharding: data-parallel over batch B=8 -> one sequence b per core.

GAT reformulation (exact, per graph g, head h):
  softmax over in-edges of dst n is invariant to any per-column scale, so
  divide the attention matrix by u[m]*v[n] (u=exp(a_s), v=exp(a_d)):
    A~[m,n] = max(rho[m]*y[n], 1) * C[m,n]
  with rho = exp(-0.8*a_s), y = exp(-0.8*a_d), C = edge-count mask.
  The row factor u[m] folds into the aggregation lhsT (xpu = xp*u, u in the
  33rd denominator slot), the column factor v[n] cancels in num/den.

  Per-tile routes (tile = [128 src x 1024 dst], 8 per (g,h)):
   R4 : R = ACT-Relu(yB*rho - 1) ; RC = R*C (DVE TT) ; PE aggregates RC and C
   R4G: same but RC on GpSimd
   R1 : A~ = DVE TS max(yB*rho, 1) ; tA = A~*C (DVE TT) ; PE aggregates tA
  Epilogue uses max_n relu(x) = relu(max_n x) to pool before bias+relu.
"""

import numpy as np

import concourse.bacc as bacc
import concourse.bass as bass
import concourse.mybir as mybir
import concourse.tile as tile
from concourse.bass_utils import run_bass_kernel_spmd

B, T, N, F_IN = 8, 16, 1000, 16
H, D = 4, 32
HD = H * D          # 128
HL = 64
OUT = 8
NPAD = 1024
NBLK = 8
G = T

FP = mybir.dt.float32
BF = mybir.dt.bfloat16
AX = mybir.AxisListType
AF = mybir.ActivationFunctionType
OPS = mybir.AluOpType

# route per (h, J): 4=R4 (DVE mask), 5=R4G (GpSimd mask), 1=R1 (DVE TS+TT)
ROUTE = [
    [4, 5, 4, 4, 1, 4, 4, 1],
    [4, 4, 5, 4, 4, 1, 4, 4],
    [4, 1, 4, 4, 5, 4, 4, 1],
    [4, 4, 4, 1, 4, 4, 5, 4],
]

_CACHE = {}


def _build_nc():
    nc = bacc.Bacc("TRN2", target_bir_lowering=False, debug=False)

    # ---- DRAM I/O ----
    d_xt = nc.dram_tensor("x_t", [F_IN, G * NPAD], BF, kind="ExternalInput").ap()
    d_wgat = nc.dram_tensor("w_gat", [F_IN, HD], BF, kind="ExternalInput").ap()
    d_was = nc.dram_tensor("w_as", [F_IN, H], BF, kind="ExternalInput").ap()
    d_wad = nc.dram_tensor("w_ad", [F_IN, H], BF, kind="ExternalInput").ap()
    d_cnt = nc.dram_tensor("cntmask", [128, NBLK * NPAD], BF, kind="ExternalInput").ap()
    d_ones = nc.dram_tensor("ones65", [65, 128], BF, kind="ExternalInput").ap()
    d_bgat = nc.dram_tensor("b_gat", [32, H], FP, kind="ExternalInput").ap()
    d_wih = nc.dram_tensor("wih_t", [HD, 4 * HL], FP, kind="ExternalInput").ap()
    d_whh = nc.dram_tensor("whh_t", [HL, 4 * HL], FP, kind="ExternalInput").ap()
    d_bls = nc.dram_tensor("b_lstm", [HL, 4], FP, kind="ExternalInput").ap()
    d_wclf = nc.dram_tensor("wclf_t", [HL, OUT], FP, kind="ExternalInput").ap()
    d_bclf = nc.dram_tensor("b_clf", [OUT, 1], FP, kind="ExternalInput").ap()
    d_y = nc.dram_tensor("y", [OUT, 1], FP, kind="ExternalOutput").ap()

    with tile.TileContext(nc) as tc:
        with (
            tc.tile_pool(name="const", bufs=1) as cpool,
            tc.tile_pool(name="stage", bufs=2) as spool,
            tc.tile_pool(name="edense", bufs=4) as epool,
            tc.tile_pool(name="small", bufs=2) as mpool,
            tc.tile_pool(name="lstm", bufs=2) as lpool,
            tc.tile_pool(name="ps_misc", bufs=2, space="PSUM") as ps_misc,
            tc.tile_pool(name="ps_out", bufs=2, space="PSUM") as ps_out,
            tc.tile_pool(name="ps_y", bufs=1, space="PSUM") as ps_y,
        ):
            # ---- constants ----
            c_xT = cpool.tile([F_IN, G * NPAD], BF, tag="xT")
            nc.sync.dma_start(c_xT[:], d_xt)
            c_wgat = cpool.tile([F_IN, HD], BF, tag="wgat")
            nc.sync.dma_start(c_wgat[:], d_wgat)
            c_was = cpool.tile([F_IN, H], BF, tag="was")
            nc.sync.dma_start(c_was[:], d_was)
            c_wad = cpool.tile([F_IN, H], BF, tag="wad")
            nc.sync.dma_start(c_wad[:], d_wad)
            c_cnt = cpool.tile([128, NBLK * NPAD], BF, tag="cnt")
            nc.sync.dma_start(c_cnt[:], d_cnt)
            c_onesB = cpool.tile([65, 128], BF, tag="onesB")
            nc.sync.dma_start(c_onesB[:], d_ones)
            c_bgat = cpool.tile([32, H], FP, tag="bgat")
            nc.sync.dma_start(c_bgat[:], d_bgat)
            c_wih = cpool.tile([HD, 4 * HL], FP, tag="wih")
            nc.sync.dma_start(c_wih[:], d_wih)
            c_whh = cpool.tile([HL, 4 * HL], FP, tag="whh")
            nc.sync.dma_start(c_whh[:], d_whh)
            c_bls = cpool.tile([HL, 4], FP, tag="bls")
            nc.sync.dma_start(c_bls[:], d_bls)
            c_wclf = cpool.tile([HL, OUT], FP, tag="wclf")
            nc.sync.dma_start(c_wclf[:], d_wclf)
            c_bclf = cpool.tile([OUT, 1], FP, tag="bclf")
            nc.sync.dma_start(c_bclf[:], d_bclf)

            c_neg1 = cpool.tile([128, 1], FP, tag="neg1")
            nc.vector.memset(c_neg1[:], -1.0)
            c_ones32 = cpool.tile([33, 32], FP, tag="ones32")
            nc.vector.memset(c_ones32[:], 1.0)
            c_pool = cpool.tile([HD, G], FP, tag="pooled")
            c_ph = []
            for h in range(H):
                ph_tile = cpool.tile([32, G], FP, tag=f"pool{h}")
                c_ph.append(ph_tile)

            for g in range(G):
                xg = c_xT[:, g * NPAD:(g + 1) * NPAD]   # [16, 1024] bf16

                # ---- per-g stage: a_s cols (u, rho), a_d rows (y), xpu33 ----
                pS = ps_misc.tile([128, 4 * NBLK], FP, tag="pm")
                for J in range(NBLK):
                    nc.tensor.matmul(
                        pS[:, J * 4:(J + 1) * 4],
                        xg[:, J * 128:(J + 1) * 128], c_was[:],
                        start=True, stop=True,
                    )
                c_u = spool.tile([128, 4 * NBLK], FP, tag="ucols")
                nc.scalar.activation(c_u[:], pS[:], AF.Exp, scale=1.0)
                c_rho = spool.tile([128, 4 * NBLK], FP, tag="rhocols")
                nc.scalar.activation(c_rho[:], pS[:], AF.Exp, scale=-0.8)

                y4 = spool.tile([4, NPAD], BF, tag="y4")
                for half in range(2):
                    pAd = ps_misc.tile([4, 512], FP, tag="pm")
                    nc.tensor.matmul(
                        pAd[:],
                        c_wad[:], xg[:, half * 512:(half + 1) * 512],
                        start=True, stop=True,
                    )
                    nc.scalar.activation(
                        y4[:, half * 512:(half + 1) * 512], pAd[:],
                        AF.Exp, scale=-0.8,
                    )
                # relocate rows to matmul-legal base partitions 0/32/64 (+spare)
                y65 = spool.tile([65, NPAD], BF, tag="y65")
                nc.sync.dma_start(y65[0:1, :], y4[0:1, :])
                nc.sync.dma_start(y65[32:33, :], y4[1:2, :])
                nc.sync.dma_start(y65[64:65, :], y4[2:3, :])
                y3x = spool.tile([1, NPAD], BF, tag="y3x")
                nc.sync.dma_start(y3x[:], y4[3:4, :])

                # xpu33: [128, J*132 + h*33 + (0..31 feats, 32 = u)]
                xpu33 = spool.tile([128, NBLK * 132], BF, tag="xpu33")
                for J in range(NBLK):
                    pX = ps_misc.tile([128, HD], FP, tag="pm")
                    nc.tensor.matmul(
                        pX[:], xg[:, J * 128:(J + 1) * 128], c_wgat[:],
                        start=True, stop=True,
                    )
                    base = J * 132
                    for h in range(H):
                        nc.vector.tensor_scalar(
                            xpu33[:, base + h * 33:base + h * 33 + 32],
                            pX[:, h * 32:(h + 1) * 32],
                            c_u[:, J * 4 + h:J * 4 + h + 1], 0.0,
                            OPS.mult, OPS.add,
                        )
                    nc.vector.tensor_copy(
                        xpu33[:, base:base + 132].rearrange(
                            "p (h q) -> p h q", q=33
                        )[:, :, 32:33],
                        c_u[:, J * 4:(J + 1) * 4].rearrange("p (h q) -> p h q", q=1),
                    )

                # ---- hot loop: per (h): yB bcast + 8 J tiles + epilogue ----
                for h in range(H):
                    if h < 3:
                        r = 32 * h
                        yrow = y65[r:r + 1, :]
                    else:
                        r = 0
                        yrow = y3x[:]
                    yB = ps_y.tile([128, NPAD], FP, tag="yB")
                    for half in range(2):
                        sl = slice(half * 512, (half + 1) * 512)
                        nc.tensor.matmul(
                            yB[:, sl], c_onesB[r:r + 1, :], yrow[:, sl],
                            start=True, stop=True,
                        )
                    oph = ps_out.tile([33, NPAD], FP, tag="oph")
                    # matmul bookkeeping: contributions per half
                    n_mm = sum(2 if ROUTE[h][J] != 1 else 1 for J in range(NBLK))
                    mm_i = 0
                    # dependency-free +C matmuls first: keeps the PE fed (and
                    # at high p-state) while Scalar/DVE produce the R*C tiles
                    for J in range(NBLK):
                        if ROUTE[h][J] == 1:
                            continue
                        cslice = c_cnt[:, J * NPAD:(J + 1) * NPAD]
                        lhs = xpu33[:, J * 132 + h * 33:J * 132 + h * 33 + 33]
                        for half in range(2):
                            sl = slice(half * 512, (half + 1) * 512)
                            nc.tensor.matmul(
                                oph[:, sl], lhs, cslice[:, sl],
                                start=(mm_i == 0), stop=(mm_i == n_mm - 1),
                            )
                        mm_i += 1
                    for J in range(NBLK):
                        rt = ROUTE[h][J]
                        rho_col = c_rho[:, J * 4 + h:J * 4 + h + 1]
                        cslice = c_cnt[:, J * NPAD:(J + 1) * NPAD]
                        lhs = xpu33[:, J * 132 + h * 33:J * 132 + h * 33 + 33]
                        if rt == 1:
                            tA = epool.tile([128, NPAD], BF, tag="tA")
                            tAh = epool.tile([128, NPAD], BF, tag="tAh")
                            nc.vector.tensor_scalar(
                                tAh[:], yB[:], rho_col, 1.0, OPS.mult, OPS.max
                            )
                            nc.vector.tensor_tensor(tA[:], tAh[:], cslice, OPS.mult)
                            rhs = tA[:]
                        else:
                            R = epool.tile([128, NPAD], BF, tag="R")
                            nc.scalar.activation(
                                R[:], yB[:], AF.Relu, bias=c_neg1[:], scale=rho_col
                            )
                            RC = epool.tile([128, NPAD], BF, tag="RC")
                            eng = nc.vector if rt == 4 else nc.gpsimd
                            eng.tensor_tensor(RC[:], R[:], cslice, OPS.mult)
                            rhs = RC[:]
                        for half in range(2):
                            sl = slice(half * 512, (half + 1) * 512)
                            nc.tensor.matmul(
                                oph[:, sl], lhs, rhs[:, sl],
                                start=(mm_i == 0), stop=(mm_i == n_mm - 1),
                            )
                        mm_i += 1

                    # ---- epilogue: r = 1/den, od = num*r, maxpool, bias+relu ----
                    denr = mpool.tile([33, NPAD], FP, tag="denr")
                    nc.scalar.copy(denr[32:33, :], oph[32:33, :])
                    den32 = mpool.tile([32, 32], FP, tag="den32")
                    nc.sync.dma_start(den32[:], denr[32:33, :])
                    rec32 = mpool.tile([32, 32], FP, tag="rec32")
                    nc.vector.reciprocal(rec32[:], den32[:])
                    rech = mpool.tile([33, NPAD], FP, tag="rech")
                    nc.sync.dma_start(rech[32:33, :], rec32[:])
                    oph16 = mpool.tile([32, NPAD], BF, tag="oph16")
                    nc.scalar.copy(oph16[:], oph[0:32, :])
                    od = mpool.tile([32, NPAD], BF, tag="od")
                    for half in range(2):
                        sl = slice(half * 512, (half + 1) * 512)
                        rb = ps_misc.tile([32, 512], FP, tag="pm")
                        nc.tensor.matmul(
                            rb[:], c_ones32[32:33, :], rech[32:33, sl],
                            start=True, stop=True,
                        )
                        nc.vector.tensor_tensor(
                            od[:, sl], oph16[:, sl], rb[:], OPS.mult
                        )
                    trout = mpool.tile([32, 1], FP, tag="trout")
                    nc.vector.tensor_reduce(trout[:], od[:, 0:N], AX.X, OPS.max)
                    nc.vector.tensor_scalar(
                        c_ph[h][:, g:g + 1], trout[:],
                        c_bgat[:, h:h + 1], 0.0, OPS.add, OPS.max,
                    )

            # assemble pooled [128, G]
            for h in range(H):
                nc.sync.dma_start(c_pool[h * 32:(h + 1) * 32, :], c_ph[h][:])

            # ---- LSTM over T steps (identical to baseline) ----
            hprev = lpool.tile([HL, 1], FP, tag="h0")
            cprev = lpool.tile([HL, 1], FP, tag="c0")
            nc.vector.memset(hprev[:], 0.0)
            nc.vector.memset(cprev[:], 0.0)
            for t in range(T):
                tga = []
                for gate in range(4):
                    psg = ps_misc.tile([HL, 1], FP, tag="pm")
                    nc.tensor.matmul(
                        psg[:], c_wih[:, gate * HL:(gate + 1) * HL],
                        c_pool[:, t:t + 1], start=True, stop=False,
                    )
                    nc.tensor.matmul(
                        psg[:], c_whh[:, gate * HL:(gate + 1) * HL],
                        hprev[:], start=False, stop=True,
                    )
                    tgt = lpool.tile([HL, 1], FP, tag=f"tg{gate}")
                    sc = 1.0 if gate == 2 else 0.5
                    nc.scalar.activation(
                        tgt[:], psg[:], AF.Tanh,
                        bias=c_bls[:, gate:gate + 1], scale=sc,
                    )
                    tga.append(tgt)
                ti, tf, tg_, to = tga
                v1 = lpool.tile([HL, 1], FP, tag="v1")
                nc.vector.scalar_tensor_tensor(
                    v1[:], tf[:], 1.0, cprev[:], OPS.add, OPS.mult
                )
                v2 = lpool.tile([HL, 1], FP, tag="v2")
                nc.vector.scalar_tensor_tensor(
                    v2[:], ti[:], 1.0, tg_[:], OPS.add, OPS.mult
                )
                cnew = lpool.tile([HL, 1], FP, tag="c0")
                nc.vector.scalar_tensor_tensor(
                    cnew[:], v1[:], 0.5, v2[:], OPS.mult, OPS.add
                )
                tcn = lpool.tile([HL, 1], FP, tag="tcn")
                nc.scalar.activation(tcn[:], cnew[:], AF.Tanh, scale=0.5)
                hnew = lpool.tile([HL, 1], FP, tag="h0")
                nc.vector.scalar_tensor_tensor(
                    hnew[:], to[:], 1.0, tcn[:], OPS.add, OPS.mult
                )
                hprev, cprev = hnew, cnew

            ps3 = ps_misc.tile([OUT, 1], FP, tag="pm")
            nc.tensor.matmul(ps3[:], c_wclf[:], hprev[:], start=True, stop=True)
            ysb = lpool.tile([OUT, 1], FP, tag="ysb")
            nc.vector.tensor_tensor(ysb[:], ps3[:], c_bclf[:], OPS.add)
            nc.sync.dma_start(d_y, ysb[:])

    nc.compile()
    return nc


def _host_prep(inputs):
    x = np.asarray(inputs["x"], dtype=np.float32)
    ei = np.asarray(inputs["edge_index"])
    W_gat = np.asarray(inputs["W_gat"], dtype=np.float32)
    att_src = np.asarray(inputs["att_src"], dtype=np.float32)
    att_dst = np.asarray(inputs["att_dst"], dtype=np.float32)
    b_gat = np.asarray(inputs["b_gat"], dtype=np.float32)
    W_ih = np.asarray(inputs["W_ih"], dtype=np.float32)
    W_hh = np.asarray(inputs["W_hh"], dtype=np.float32)
    b_ih = np.asarray(inputs["b_ih"], dtype=np.float32)
    b_hh = np.asarray(inputs["b_hh"], dtype=np.float32)
    W_clf = np.asarray(inputs["W_clf"], dtype=np.float32)
    b_clf = np.asarray(inputs["b_clf"], dtype=np.float32)

    bf16 = mybir.dt.np(BF)

    Wr = W_gat.reshape(F_IN, H, D)
    W_as = np.einsum("fhd,hd->fh", Wr, att_src)
    W_ad = np.einsum("fhd,hd->fh", Wr, att_dst)

    src = ei[0].astype(np.int64)
    dst = ei[1].astype(np.int64)
    Cm = np.zeros((NPAD, NPAD), dtype=np.float32)
    np.add.at(Cm, (src, dst), 1.0)
    Cm[np.arange(N), np.arange(N)] += 1.0
    Cm[NPAD - 1, N:] = 1.0
    cntmask = (
        Cm.reshape(NBLK, 128, NPAD).transpose(1, 0, 2).reshape(128, NBLK * NPAD)
    ).astype(bf16)

    xpad = np.zeros((B, T, NPAD, F_IN), dtype=np.float32)
    xpad[:, :, :N, :] = x
    # [F, T*NPAD] per core
    xtcore = [
        np.ascontiguousarray(
            xpad[b].reshape(T * NPAD, F_IN).T
        ).astype(bf16)
        for b in range(B)
    ]

    b_gates = (b_ih + b_hh).astype(np.float32)
    bls = np.zeros((HL, 4), dtype=np.float32)
    bls[:, 0] = 0.5 * b_gates[0:64]
    bls[:, 1] = 0.5 * b_gates[64:128]
    bls[:, 2] = b_gates[128:192]
    bls[:, 3] = 0.5 * b_gates[192:256]

    common = {
        "w_gat": W_gat.astype(bf16),
        "w_as": W_as.astype(bf16),
        "w_ad": W_ad.astype(bf16),
        "cntmask": cntmask,
        "ones65": np.ones((65, 128), dtype=bf16),
        "b_gat": np.ascontiguousarray(b_gat.reshape(H, 32).T),
        "wih_t": np.ascontiguousarray(W_ih.T),
        "whh_t": np.ascontiguousarray(0.5 * W_hh.T),
        "b_lstm": bls,
        "wclf_t": np.ascontiguousarray(0.5 * W_clf.T),
        "b_clf": b_clf.reshape(OUT, 1),
    }
    in_maps = []
    for b in range(B):
        m = dict(common)
        m["x_t"] = xtcore[b]
        in_maps.append(m)
    return in_maps


def kernel(**inputs):
    if "nc" not in _CACHE:
        _CACHE["nc"] = _build_nc()
    nc = _CACHE["nc"]
    in_maps = _host_prep(inputs)
    res = run_bass_kernel_spmd(nc, in_maps, core_ids=list(range(B)))
    y = np.stack([r["y"][:, 0] for r in res.results], axis=0)
    return y.astype(np.float32)


if __name__ == "__main__":
    import reference as R

    inp = R.setup_inputs()
    inp = {k: np.asarray(v) for k, v in inp.items()}
    out = kernel(**inp)
    print(out)
